# revision 11
# baseline (speedup 1.0000x reference)
"""GAT kernel for Trainium2, SPMD over 8 NeuronCores.

Math: this GAT variant's attention logits e[b,h,i,j] do NOT depend on j
(the "untransposed Wh2" formulation), so softmax over a row whose support
(adj!=0) carries a constant value collapses to 1/deg(i) on the support and
0 elsewhere.  Per batch b:

    out[b] = elu( diag(1/deg_b) @ (adj_b * adj_weight_b) @ (h_b @ W) )

Head-independent; `a` is unused.  Sharding: data-parallel over batch
(B == n_cores == 8).

v3 schedule (per core), all fp16 matmuls (fp8-DR measured 3.6e-2 rel err
numerically > 2e-2 gate):
  - Host precomputes MT''[j,i] = (64*r_i)*(adj*adj_weight)[i,j] fp16 with
    r = 1/(32*deg) (32 = W pre-scale, 64 keeps MT'' in fp16-normal range)
    so the device applies NO per-row scaling at all.  The evac of Wh
    divides by 64 (free: ACT Copy scale / DVE tensor_scalar mul).
  - Device computes q = elu(y) + 1 = min(exp(y), max(y+1, 1)); the host
    subtracts the 1 after gathering.  This makes the combine a plain
    2-source tensor_tensor MIN (DVE 2x-capable) and the linear side a
    single tensor_scalar (add 1, max 1), vs the baseline's 3 ops with a
    scalar_tensor_tensor (1x-only).
  - All DRAM tensors partition-major; input DMA chunks sized so the first
    matmul's data (h-d0 + W-d0-f0, 256KB) lands ~3us after issue, with W
    on the sync HWDGE ring and h on the scalar HWDGE ring in parallel.
  - Junk warmup matmuls burn the HAM 1.2GHz window during the DMA wait.
  - MM1 d-outer; d0 runs f0-pass-then-f1-pass (so only W-d0-f0 gates the
    start), d1..d7 run (i, f0),(i, f1) pairs sharing the stationary
    hT[d,i] (measured 216ns/MM vs 259 for the unpaired baseline).
  - ps1/ps2 are [128,1024] 2-bank PSUM tiles -> 1024-wide epilogue ops.
  - MM2 i-outer; epilogue per i overlaps the next i's matmuls; per-i
    256KB output DMA (2KB descriptors); last tile half-split.
"""

import os

import numpy as np

import concourse.bass as bass
import concourse.tile as tile
from concourse import bacc, mybir
from concourse.bass import ts
from concourse.bass_utils import run_bass_kernel_spmd

B, N, D = 8, 512, 1024
P = 128  # SBUF partitions
NB = N // P  # 4 row blocks (i / j)
DB = D // P  # 8 contraction blocks (d)
WSCALE = 32.0  # W pre-scale
MSCALE = 64.0  # MT pre-scale (folded out during evac)
NWARM = 28  # junk matmuls burning the HAM window (N=128 each)

F32 = mybir.dt.float32
F16 = mybir.dt.float16
AF = mybir.ActivationFunctionType
ALU = mybir.AluOpType


def build_nc():
    nc = bacc.Bacc("TRN2", target_bir_lowering=False, debug=False, num_devices=B)

    # partition-major DRAM layouts (host packs/unpacks):
    #   hp[p, d, i] = h[i, 128d+p]          (h transposed)
    #   Wp[p, d, f] = 32*W[128d+p, f]
    #   Mp[p, j, i] = (64*r_i) * (adj*adj_weight)[i, 128j+p]
    #   op[p, ib, f] = elu(out)[128ib+p, f] + 1
    hp = nc.dram_tensor("hp", [P, DB, N], F16, kind="ExternalInput").ap()
    Wp = nc.dram_tensor("Wp", [P, DB, D], F16, kind="ExternalInput").ap()
    Mp = nc.dram_tensor("Mp", [P, NB, N], F16, kind="ExternalInput").ap()
    op = nc.dram_tensor("op", [P, NB, D], F16, kind="ExternalOutput").ap()

    with tile.TileContext(nc) as tc:
        with (
            tc.tile_pool(name="singles", bufs=1) as singles,
            tc.tile_pool(name="work", bufs=2) as work,
            tc.tile_pool(name="outp", bufs=2) as outp,
            tc.tile_pool(name="psum", bufs=4, space="PSUM") as psum,
        ):
            hT_sb = singles.tile([P, DB, N], F16)   # [p, d, i] 1 MB
            W_sb = singles.tile([P, DB, D], F16)    # [p, d, f] 2 MB
            MT_sb = singles.tile([P, NB, N], F16)   # [p, j, i] 512 KB
            Wh_sb = singles.tile([P, NB, D], F16)   # [p, j, f] 1 MB
            junk = singles.tile([P, 512], F16)      # warmup fodder

            # ---- all inputs on ONE ring (sync), in consumption order -----
            # (two competing rings round-robin at the SDMA level, which let
            # slack-rich chunks starve the critical W stream — measured the
            # W ring at 66 GB/s while h/MT hogged 295.)
            nc.sync.dma_start(hT_sb[:, 0], hp[:, 0])           # 128 KB
            nc.sync.dma_start(W_sb[:, 0], Wp[:, 0])            # 256 KB

            # ---- PE warmup (junk output, never read back) -----------------
            # many small back-to-back matmuls: keeps the HAM activity
            # window gap-free until real data lands, and the last one only
            # delays MM1 by ~0.15us.
            nc.vector.memset(junk[:, 0:P], 0.0)
            warm_ps = psum.tile([P, D], F32, tag="mm")
            for k in range(NWARM):
                nc.tensor.matmul(
                    warm_ps[:, 512 * (k % 2) : 512 * (k % 2) + P],
                    junk[:, :P], junk[:, :P],
                    start=True, stop=True,
                )

            # ---- remaining input DMAs, consumption order -----------------
            nc.sync.dma_start(hT_sb[:, 1], hp[:, 1])           # 128 KB
            nc.sync.dma_start(W_sb[:, 1], Wp[:, 1])            # 256 KB
            nc.sync.dma_start(hT_sb[:, 2:4], hp[:, 2:4])       # 256 KB
            nc.sync.dma_start(W_sb[:, 2], Wp[:, 2])            # 256 KB
            nc.sync.dma_start(W_sb[:, 3], Wp[:, 3])            # 256 KB
            nc.sync.dma_start(hT_sb[:, 4:6], hp[:, 4:6])       # 256 KB
            nc.sync.dma_start(W_sb[:, 4:6], Wp[:, 4:6])        # 512 KB
            nc.sync.dma_start(hT_sb[:, 6:8], hp[:, 6:8])       # 256 KB
            nc.sync.dma_start(W_sb[:, 6:8], Wp[:, 6:8])        # 512 KB
            nc.sync.dma_start(MT_sb, Mp)                       # 512 KB

            # ---- MM1: Wh = hT.T @ W ---------------------------------------
            # (i, f0),(i, f1) pairs share the stationary hT[d,i]: measured
            # 216ns/MM vs 259 unpaired.  Phase A (d0-d4) is DMA-paced over
            # all i; phase B finishes each i's last 3 d-blocks in turn so
            # the Wh banks complete staggered and their evacs overlap the
            # rest of MM1 (instead of all serializing after it).
            ps1 = [
                psum.tile([P, D], F32, name=f"ps1_{i}", tag="mm")
                for i in range(NB)
            ]
            DSPLIT = 5
            for d in range(DSPLIT):
                for i in range(NB):
                    lhsT = hT_sb[:, d, ts(i, P)]
                    nc.tensor.matmul(
                        ps1[i][:, 0:512], lhsT, W_sb[:, d, 0:512],
                        start=(d == 0), stop=False,
                    )
                    nc.tensor.matmul(
                        ps1[i][:, 512:1024], lhsT, W_sb[:, d, 512:1024],
                        start=(d == 0), stop=False,
                    )
            for i in range(NB):
                for d in range(DSPLIT, DB):
                    lhsT = hT_sb[:, d, ts(i, P)]
                    nc.tensor.matmul(
                        ps1[i][:, 0:512], lhsT, W_sb[:, d, 0:512],
                        start=False, stop=(d == DB - 1),
                    )
                    nc.tensor.matmul(
                        ps1[i][:, 512:1024], lhsT, W_sb[:, d, 512:1024],
                        start=False, stop=(d == DB - 1),
                    )
                # evac ps1_i -> Wh fp16 (/64) as soon as bank i completes;
                # the last bank is half-split across both engines so the
                # MM1->MM2 gap is one 512-wide op.
                if i < NB - 1:
                    if i % 2 == 0:
                        nc.scalar.activation(
                            Wh_sb[:, i], ps1[i], AF.Copy, scale=1.0 / MSCALE
                        )
                    else:
                        nc.vector.tensor_scalar_mul(
                            Wh_sb[:, i], ps1[i], 1.0 / MSCALE
                        )
                else:
                    nc.scalar.activation(
                        Wh_sb[:, i, 0:512], ps1[i][:, 0:512],
                        AF.Copy, scale=1.0 / MSCALE,
                    )
                    nc.vector.tensor_scalar_mul(
                        Wh_sb[:, i, 512:1024], ps1[i][:, 512:1024], 1.0 / MSCALE
                    )

            # ---- MM2 + epilogue, i-outer ---------------------------------
            for i in range(NB):
                ps2 = psum.tile([P, D], F32, name=f"ps2_{i}", tag="mm")
                for j in range(NB):
                    lhsT = MT_sb[:, j, ts(i, P)]
                    nc.tensor.matmul(
                        ps2[:, 0:512], lhsT, Wh_sb[:, j, 0:512],
                        start=(j == 0), stop=(j == NB - 1),
                    )
                    nc.tensor.matmul(
                        ps2[:, 512:1024], lhsT, Wh_sb[:, j, 512:1024],
                        start=(j == 0), stop=(j == NB - 1),
                    )

                # q = min(exp(y), max(y+1, 1)) = elu(y) + 1  (host does -1)
                exp_t = work.tile([P, D], F16, tag="exp")
                lin_t = work.tile([P, D], F16, tag="lin")
                o_t = outp.tile([P, D], F16)
                if i < NB - 1:
                    nc.scalar.activation(exp_t, ps2, AF.Exp)
                    nc.vector.tensor_scalar(
                        lin_t, ps2, 1.0, 1.0, op0=ALU.add, op1=ALU.max
                    )
                    nc.vector.tensor_tensor(o_t, exp_t, lin_t, op=ALU.min)
                    eng = nc.scalar if i % 2 == 0 else nc.sync
                    eng.dma_start(op[:, i], o_t)
                else:
                    # half-split the last tile so its DMA starts early
                    for hh in range(2):
                        sl = slice(hh * 512, (hh + 1) * 512)
                        nc.scalar.activation(exp_t[:, sl], ps2[:, sl], AF.Exp)
                        nc.vector.tensor_scalar(
                            lin_t[:, sl], ps2[:, sl], 1.0, 1.0,
                            op0=ALU.add, op1=ALU.max,
                        )
                        nc.vector.tensor_tensor(
                            o_t[:, sl], exp_t[:, sl], lin_t[:, sl], op=ALU.min
                        )
                        eng = nc.scalar if hh == 0 else nc.sync
                        eng.dma_start(op[:, i, sl], o_t[:, sl])

    nc.compile()
    return nc


_NC = None


def _get_nc():
    global _NC
    if _NC is None:
        _NC = build_nc()
    return _NC


def _part_major(x, nb):
    """[nb*128, R] -> [128, nb, R] partition-major."""
    n, r = x.shape
    return np.ascontiguousarray(x.reshape(nb, P, r).transpose(1, 0, 2))


def _in_maps(h, adj, adj_weight, W):
    h = np.asarray(h, dtype=np.float32)
    adj = np.asarray(adj)
    adjw = np.asarray(adj_weight, dtype=np.float32)
    Wf = np.asarray(W, dtype=np.float32).reshape(D, D)
    Wp = _part_major((Wf * WSCALE).astype(np.float16), DB)      # [128, 8, 1024]
    deg = adj.sum(axis=2).astype(np.float32)                     # [B, 512]
    r = MSCALE / (WSCALE * deg)                                  # [B, 512]
    M = (adj * adjw * r[:, :, None]).astype(np.float16)          # [B, 512, 512]
    maps = []
    for b in range(B):
        hT = np.ascontiguousarray(h[b].T).astype(np.float16)     # [1024, 512]
        MT = np.ascontiguousarray(M[b].T)                        # [512, 512]
        maps.append(
            {
                "hp": _part_major(hT, DB),                       # [128, 8, 512]
                "Wp": Wp,
                "Mp": _part_major(MT, NB),                       # [128, 4, 512]
            }
        )
    return maps


def _run(h, adj, adj_weight, W, a=None, trace=False, **trace_kw):
    nc = _get_nc()
    res = run_bass_kernel_spmd(
        nc, _in_maps(h, adj, adj_weight, W), core_ids=list(range(B)),
        trace=trace, **trace_kw,
    )
    # op [128, 4, 1024] (elu+1) -> out [512, 1024]
    out = np.stack(
        [
            np.asarray(res.results[c]["op"])
            .transpose(1, 0, 2)
            .reshape(N, D)
            .astype(np.float32)
            for c in range(B)
        ],
        axis=0,
    )
    return out - 1.0, res


def kernel(h, adj, adj_weight, W, a=None, **_ignored):
    # The NTFF trace path needs an axon hook module this container lacks;
    # make sure an ambient BASS_TRACE can't divert the graded run into it.
    os.environ["BASS_NEVER_TRACE"] = "1"
    out, _ = _run(h, adj, adj_weight, W)
    return out


# revision 15
# speedup vs baseline: 1.0559x; 1.0559x over previous
"""GAT kernel for Trainium2, SPMD over 8 NeuronCores.

Math: this GAT variant's attention logits e[b,h,i,j] do NOT depend on j
(the "untransposed Wh2" formulation), so softmax over a row whose support
(adj!=0) carries a constant value collapses to 1/deg(i) on the support and
0 elsewhere.  Per batch b:

    out[b] = elu( diag(1/deg_b) @ (adj_b * adj_weight_b) @ (h_b @ W) )

Head-independent; `a` is unused.  Sharding: data-parallel over batch
(B == n_cores == 8).

v3 schedule (per core), all fp16 matmuls (fp8-DR measured 3.6e-2 rel err
numerically > 2e-2 gate):
  - Host precomputes MT''[j,i] = (64*r_i)*(adj*adj_weight)[i,j] fp16 with
    r = 1/(32*deg) (32 = W pre-scale, 64 keeps MT'' in fp16-normal range)
    so the device applies NO per-row scaling at all.  The evac of Wh
    divides by 64 (free: ACT Copy scale / DVE tensor_scalar mul).
  - Device computes q = elu(y) + 1 = min(exp(y), max(y+1, 1)); the host
    subtracts the 1 after gathering.  This makes the combine a plain
    2-source tensor_tensor MIN (DVE 2x-capable) and the linear side a
    single tensor_scalar (add 1, max 1), vs the baseline's 3 ops with a
    scalar_tensor_tensor (1x-only).
  - All DRAM tensors partition-major; input DMA chunks sized so the first
    matmul's data (h-d0 + W-d0-f0, 256KB) lands ~3us after issue, with W
    on the sync HWDGE ring and h on the scalar HWDGE ring in parallel.
  - Junk warmup matmuls burn the HAM 1.2GHz window during the DMA wait.
  - MM1 d-outer; d0 runs f0-pass-then-f1-pass (so only W-d0-f0 gates the
    start), d1..d7 run (i, f0),(i, f1) pairs sharing the stationary
    hT[d,i] (measured 216ns/MM vs 259 for the unpaired baseline).
  - ps1/ps2 are [128,1024] 2-bank PSUM tiles -> 1024-wide epilogue ops.
  - MM2 i-outer; epilogue per i overlaps the next i's matmuls; per-i
    256KB output DMA (2KB descriptors); last tile half-split.
"""

import os

import numpy as np

import concourse.bass as bass
import concourse.tile as tile
from concourse import bacc, mybir
from concourse.bass import ts
from concourse.bass_utils import run_bass_kernel_spmd

B, N, D = 8, 512, 1024
P = 128  # SBUF partitions
NB = N // P  # 4 row blocks (i / j)
DB = D // P  # 8 contraction blocks (d)
WSCALE = 32.0  # W pre-scale
MSCALE = 64.0  # MT pre-scale (folded out during evac)
NWARM = 28  # junk matmuls burning the HAM window (N=128 each)

F32 = mybir.dt.float32
F16 = mybir.dt.float16
AF = mybir.ActivationFunctionType
ALU = mybir.AluOpType


def build_nc():
    nc = bacc.Bacc("TRN2", target_bir_lowering=False, debug=False, num_devices=B)

    # partition-major DRAM layouts (host packs/unpacks):
    #   hwp[p, d, 0:512]    = h[:, 128d+p]   (h transposed)
    #   hwp[p, d, 512:1536] = 32*W[128d+p, :]
    #   Mp[p, j, i] = (64*r_i) * (adj*adj_weight)[i, 128j+p]
    #   op[p, ib, f] = elu(out)[128ib+p, f] + 1
    # h and W share one tensor so each MM1 d-block is ONE 384KB DMA with
    # 3KB-contiguous descriptors, sem granularity == consumption
    # granularity, and no cross-ring bandwidth competition.
    hwp = nc.dram_tensor("hwp", [P, DB, N + D], F16, kind="ExternalInput").ap()
    Mp = nc.dram_tensor("Mp", [P, NB, N], F16, kind="ExternalInput").ap()
    op = nc.dram_tensor("op", [P, NB, D], F16, kind="ExternalOutput").ap()

    with tile.TileContext(nc) as tc:
        with (
            tc.tile_pool(name="singles", bufs=1) as singles,
            tc.tile_pool(name="work", bufs=2) as work,
            tc.tile_pool(name="outp", bufs=2) as outp,
            tc.tile_pool(name="psum", bufs=4, space="PSUM") as psum,
        ):
            hw_sb = singles.tile([P, DB, N + D], F16)  # [p, d, hT|W] 3 MB
            MT_sb = singles.tile([P, NB, N], F16)   # [p, j, i] 512 KB
            Wh_sb = singles.tile([P, NB, D], F16)   # [p, j, f] 1 MB
            junk = singles.tile([P, 512], F16)      # warmup fodder

            # ---- all inputs on ONE ring (sync), in consumption order -----
            # (two competing rings round-robin at the SDMA level, which let
            # slack-rich chunks starve the critical W stream — measured the
            # W ring at 66 GB/s while h/MT hogged 295.)
            nc.sync.dma_start(hw_sb[:, 0], hwp[:, 0])          # 384 KB

            # ---- PE warmup (junk output, never read back) -----------------
            # many small back-to-back matmuls: keeps the HAM activity
            # window gap-free until real data lands, and the last one only
            # delays MM1 by ~0.15us.
            nc.vector.memset(junk[:, 0:P], 0.0)
            warm_ps = psum.tile([P, D], F32, tag="mm")
            for k in range(NWARM):
                nc.tensor.matmul(
                    warm_ps[:, 512 * (k % 2) : 512 * (k % 2) + P],
                    junk[:, :P], junk[:, :P],
                    start=True, stop=True,
                )

            # ---- remaining input DMAs, consumption order -----------------
            for d in range(1, DB):
                nc.sync.dma_start(hw_sb[:, d], hwp[:, d])      # 384 KB each
            nc.sync.dma_start(MT_sb, Mp)                       # 512 KB

            # ---- MM1: Wh = hT.T @ W ---------------------------------------
            # (i, f0),(i, f1) pairs share the stationary hT[d,i]: measured
            # 216ns/MM vs 259 unpaired.  Phase A (d0-d4) is DMA-paced over
            # all i; phase B finishes each i's last 3 d-blocks in turn so
            # the Wh banks complete staggered and their evacs overlap the
            # rest of MM1 (instead of all serializing after it).
            ps1 = [
                psum.tile([P, D], F32, name=f"ps1_{i}", tag="mm")
                for i in range(NB)
            ]
            DSPLIT = 5
            for d in range(DSPLIT):
                for i in range(NB):
                    lhsT = hw_sb[:, d, ts(i, P)]
                    nc.tensor.matmul(
                        ps1[i][:, 0:512], lhsT, hw_sb[:, d, 512:1024],
                        start=(d == 0), stop=False,
                    )
                    nc.tensor.matmul(
                        ps1[i][:, 512:1024], lhsT, hw_sb[:, d, 1024:1536],
                        start=(d == 0), stop=False,
                    )
            for i in range(NB):
                for d in range(DSPLIT, DB):
                    lhsT = hw_sb[:, d, ts(i, P)]
                    nc.tensor.matmul(
                        ps1[i][:, 0:512], lhsT, hw_sb[:, d, 512:1024],
                        start=False, stop=(d == DB - 1),
                    )
                    nc.tensor.matmul(
                        ps1[i][:, 512:1024], lhsT, hw_sb[:, d, 1024:1536],
                        start=False, stop=(d == DB - 1),
                    )
                # evac ps1_i -> Wh fp16 (/64) as soon as bank i completes;
                # the last bank is half-split across both engines so the
                # MM1->MM2 gap is one 512-wide op.
                if i < NB - 1:
                    if i % 2 == 0:
                        nc.scalar.activation(
                            Wh_sb[:, i], ps1[i], AF.Copy, scale=1.0 / MSCALE
                        )
                    else:
                        nc.vector.tensor_scalar_mul(
                            Wh_sb[:, i], ps1[i], 1.0 / MSCALE
                        )
                else:
                    nc.scalar.activation(
                        Wh_sb[:, i, 0:512], ps1[i][:, 0:512],
                        AF.Copy, scale=1.0 / MSCALE,
                    )
                    nc.vector.tensor_scalar_mul(
                        Wh_sb[:, i, 512:1024], ps1[i][:, 512:1024], 1.0 / MSCALE
                    )

            # ---- MM2 + epilogue, i-outer ---------------------------------
            for i in range(NB):
                ps2 = psum.tile([P, D], F32, name=f"ps2_{i}", tag="mm")
                for j in range(NB):
                    lhsT = MT_sb[:, j, ts(i, P)]
                    nc.tensor.matmul(
                        ps2[:, 0:512], lhsT, Wh_sb[:, j, 0:512],
                        start=(j == 0), stop=(j == NB - 1),
                    )
                    nc.tensor.matmul(
                        ps2[:, 512:1024], lhsT, Wh_sb[:, j, 512:1024],
                        start=(j == 0), stop=(j == NB - 1),
                    )

                # q = min(exp(y), max(y+1, 1)) = elu(y) + 1  (host does -1)
                exp_t = work.tile([P, D], F16, tag="exp")
                lin_t = work.tile([P, D], F16, tag="lin")
                o_t = outp.tile([P, D], F16)
                if i < NB - 1:
                    nc.scalar.activation(exp_t, ps2, AF.Exp)
                    nc.vector.tensor_scalar(
                        lin_t, ps2, 1.0, 1.0, op0=ALU.add, op1=ALU.max
                    )
                    nc.vector.tensor_tensor(o_t, exp_t, lin_t, op=ALU.min)
                    eng = nc.scalar if i % 2 == 0 else nc.sync
                    eng.dma_start(op[:, i], o_t)
                else:
                    # half-split the last tile so its DMA starts early
                    for hh in range(2):
                        sl = slice(hh * 512, (hh + 1) * 512)
                        nc.scalar.activation(exp_t[:, sl], ps2[:, sl], AF.Exp)
                        nc.vector.tensor_scalar(
                            lin_t[:, sl], ps2[:, sl], 1.0, 1.0,
                            op0=ALU.add, op1=ALU.max,
                        )
                        nc.vector.tensor_tensor(
                            o_t[:, sl], exp_t[:, sl], lin_t[:, sl], op=ALU.min
                        )
                        eng = nc.scalar if hh == 0 else nc.sync
                        eng.dma_start(op[:, i, sl], o_t[:, sl])

    nc.compile()
    return nc


_NC = None


def _get_nc():
    global _NC
    if _NC is None:
        _NC = build_nc()
    return _NC


def _part_major(x, nb):
    """[nb*128, R] -> [128, nb, R] partition-major."""
    n, r = x.shape
    return np.ascontiguousarray(x.reshape(nb, P, r).transpose(1, 0, 2))


def _in_maps(h, adj, adj_weight, W):
    h = np.asarray(h, dtype=np.float32)
    adj = np.asarray(adj)
    adjw = np.asarray(adj_weight, dtype=np.float32)
    Wf = np.asarray(W, dtype=np.float32).reshape(D, D)
    Wpm = _part_major((Wf * WSCALE).astype(np.float16), DB)      # [128, 8, 1024]
    deg = adj.sum(axis=2).astype(np.float32)                     # [B, 512]
    r = MSCALE / (WSCALE * deg)                                  # [B, 512]
    M = (adj * adjw * r[:, :, None]).astype(np.float16)          # [B, 512, 512]
    maps = []
    for b in range(B):
        hT = np.ascontiguousarray(h[b].T).astype(np.float16)     # [1024, 512]
        hpm = _part_major(hT, DB)                                # [128, 8, 512]
        hwp = np.ascontiguousarray(
            np.concatenate([hpm, Wpm], axis=2)                   # [128, 8, 1536]
        )
        MT = np.ascontiguousarray(M[b].T)                        # [512, 512]
        maps.append(
            {
                "hwp": hwp,
                "Mp": _part_major(MT, NB),                       # [128, 4, 512]
            }
        )
    return maps


def _run(h, adj, adj_weight, W, a=None, trace=False, **trace_kw):
    nc = _get_nc()
    res = run_bass_kernel_spmd(
        nc, _in_maps(h, adj, adj_weight, W), core_ids=list(range(B)),
        trace=trace, **trace_kw,
    )
    # op [128, 4, 1024] (elu+1) -> out [512, 1024]
    out = np.stack(
        [
            np.asarray(res.results[c]["op"])
            .transpose(1, 0, 2)
            .reshape(N, D)
            .astype(np.float32)
            for c in range(B)
        ],
        axis=0,
    )
    return out - 1.0, res


def kernel(h, adj, adj_weight, W, a=None, **_ignored):
    # The NTFF trace path needs an axon hook module this container lacks;
    # make sure an ambient BASS_TRACE can't divert the graded run into it.
    os.environ["BASS_NEVER_TRACE"] = "1"
    out, _ = _run(h, adj, adj_weight, W)
    return out
